# revision 1
# baseline (speedup 1.0000x reference)
"""Causal multi-head attention (B=4, T=2048, D=2048, H=16) on 8 Trainium2
NeuronCores via Bass/Tile, SPMD with zero collectives.

Sharding: each batch b is split over two cores by query rows using a
zigzag quarter split (core 2b: quarters Q1+Q4, core 2b+1: quarters Q2+Q3),
which balances the causal-attention triangle. Every core computes K/V
projections for its batch's full sequence (cheap redundancy that keeps the
SPMD program uniform across cores); causal masking is supplied as per-core
input data over a uniform tile pattern, so all 8 cores run the same
instruction stream.

Per-core pipeline (all matmuls in float32r — full PE rate, ~1e-4 rel err):
  0. PE-transpose x -> xT (SBUF slabs, one T/2 half at a time)
  1. K^T = Wk^T xT, Q^T = Wq^T xT (transposed layouts), V = x Wv (natural)
  2. per head: S^T tiles = K^T_chunk^T Q^T, exp on ACT (no max subtraction:
     scores are O(1) by construction), causal/pad masking by DVE multiply,
     A^T accumulated on PE with V as stationary operand, softmax denominators
     via ones-vector matmuls, normalization fused into the PSUM evacuation
  3. O rows = A^T^T Wo + bo
Outputs are the core's own (permuted) query rows; the host scatters them
back into the full [B, T, D] tensor.
"""
import numpy as np

import concourse.bacc as bacc
import concourse.mybir as mybir
from concourse.tile import TileContext
from concourse.bass_utils import run_bass_kernel_spmd

F32 = mybir.dt.float32
F32R = mybir.dt.float32r
EXP = mybir.ActivationFunctionType.Exp
MULT = mybir.AluOpType.mult

PROD_CFG = dict(B=4, T=2048, D=2048, H=16)
PIPELINE = True


def _derived(cfg):
    B, T, D, H = cfg["B"], cfg["T"], cfg["D"], cfg["H"]
    d = dict(cfg)
    d.update(
        QW=T // 4,            # quarter width (query-row shard unit)
        OWN=T // 2,           # own query rows per core
        T2=T // 2,            # xT slab half width
        DK=D // 128,          # contraction chunks
        q=T // 4 // 128,      # 128-row j-tiles per quarter
        NCH=min(512, T // 2),  # moving-N chunk for projections
        ND=min(512, D),       # phase-4 output-column slab width
        DH=128,
        N_CORES=2 * B,
    )
    return d


def _r(ap):
    return ap.bitcast(F32R)


def build_nc(cfg):
    c = _derived(cfg)
    T, D, H = c["T"], c["D"], c["H"]
    QW, OWN, T2, DK, q = c["QW"], c["OWN"], c["T2"], c["DK"], c["q"]
    NCH, ND = c["NCH"], c["ND"]
    SCALE = float(c["DH"] ** -0.5)

    nc = bacc.Bacc(
        "TRN2", target_bir_lowering=False, debug=False, num_devices=c["N_CORES"]
    )
    x = nc.dram_tensor("x", [T, D], F32R, kind="ExternalInput").ap()
    wq = nc.dram_tensor("wq", [D, D], F32R, kind="ExternalInput").ap()
    wk = nc.dram_tensor("wk", [D, D], F32R, kind="ExternalInput").ap()
    wv = nc.dram_tensor("wv", [D, D], F32R, kind="ExternalInput").ap()
    wo = nc.dram_tensor("wo", [D, D], F32R, kind="ExternalInput").ap()
    bq = nc.dram_tensor("bq", [D], F32, kind="ExternalInput").ap()
    bk = nc.dram_tensor("bk", [D], F32, kind="ExternalInput").ap()
    bv = nc.dram_tensor("bv", [D], F32, kind="ExternalInput").ap()
    bo = nc.dram_tensor("bo", [D], F32, kind="ExternalInput").ap()
    mask = nc.dram_tensor("mask", [128, 4 * q * QW], F32R, kind="ExternalInput").ap()
    ident_in = nc.dram_tensor("ident", [128, 128], F32R, kind="ExternalInput").ap()
    ones_c_in = nc.dram_tensor("ones_c", [128, 1], F32R, kind="ExternalInput").ap()
    ones_r_in = nc.dram_tensor("ones_r", [1, 128], F32R, kind="ExternalInput").ap()
    o = nc.dram_tensor("o", [OWN, D], F32, kind="ExternalOutput").ap()

    kt_d = nc.dram_tensor("kt_scratch", [D, T], F32R).ap()
    qt_d = nc.dram_tensor("qt_scratch", [D, OWN], F32R).ap()
    v_d = nc.dram_tensor("v_scratch", [T, D], F32R).ap()

    # uniform causal j-tile windows (see module docstring)
    LWIN = list(range(q)) + list(range(2 * q, 3 * q))          # L+H valid
    HONLY = list(range(q, 2 * q)) + list(range(3 * q, 4 * q))  # H valid only

    with TileContext(nc) as tc:
        with (
            tc.tile_pool(name="const", bufs=1) as pconst,
        ):
            ident = pconst.tile([128, 128], F32R, tag="ident")
            nc.sync.dma_start(out=ident[:], in_=ident_in[:])
            ones_col = pconst.tile([128, 1], F32R, tag="ones_col")
            nc.sync.dma_start(out=ones_col[:], in_=ones_c_in[:])
            ones_row = pconst.tile([1, 128], F32R, tag="ones_row")
            nc.sync.dma_start(out=ones_row[:], in_=ones_r_in[:])
            bk_sb = pconst.tile([128, DK], F32, tag="bk")
            nc.sync.dma_start(out=bk_sb[:], in_=bk.rearrange("(m p) -> p m", p=128))
            bq_sb = pconst.tile([128, DK], F32, tag="bq")
            nc.sync.dma_start(out=bq_sb[:], in_=bq.rearrange("(m p) -> p m", p=128))
            bv_sb = pconst.tile([1, D], F32R, tag="bv")
            nc.sync.dma_start(out=bv_sb[:], in_=bv[None, :].bitcast(F32R))
            bo_sb = pconst.tile([1, D], F32R, tag="bo")
            nc.sync.dma_start(out=bo_sb[:], in_=bo[None, :].bitcast(F32R))

            # ---------------- phase 0+1: xT, K^T, Q^T, V ----------------
            with (
                tc.tile_pool(name="slab", bufs=1) as pslab,
                tc.tile_pool(name="p1x", bufs=2) as p1x,
                tc.tile_pool(name="p1w", bufs=2) as p1w,
                tc.tile_pool(name="p1wv", bufs=2) as p1wv,
                tc.tile_pool(name="p1st", bufs=3) as p1st,
                tc.tile_pool(name="ps_tr", bufs=2, space="PSUM") as ps_tr,
                tc.tile_pool(name="ps_kq", bufs=2, space="PSUM") as ps_kq,
                tc.tile_pool(name="ps_v", bufs=2, space="PSUM") as ps_v,
            ):
                for hf in range(2):
                    slab = pslab.tile([128, DK * T2], F32R, tag="slab")
                    slab3 = slab[:].rearrange("p (k t) -> p k t", k=DK)
                    # transpose x rows [hf*T2, (hf+1)*T2) into slab
                    for tcn in range(T2 // 128):
                        xst = p1x.tile([128, D], F32R, tag="xst")
                        nc.sync.dma_start(
                            out=xst[:],
                            in_=x[hf * T2 + tcn * 128: hf * T2 + (tcn + 1) * 128, :],
                        )
                        for kb in range(0, DK, 4):
                            nb = min(4, DK - kb)
                            ps = ps_tr.tile([128, 512], F32R, tag="pstr")
                            for i in range(nb):
                                nc.tensor.transpose(
                                    ps[:, i * 128:(i + 1) * 128],
                                    xst[:, (kb + i) * 128:(kb + i + 1) * 128],
                                    ident[:],
                                )
                            nc.vector.tensor_copy(
                                slab3[:, kb:kb + nb, tcn * 128:(tcn + 1) * 128],
                                ps[:, : nb * 128].rearrange(
                                    "p (a b) -> p a b", a=nb
                                ),
                            )
                    # K^T (and Q^T on half 0) projections
                    projs = [(wk, bk_sb, kt_d, True)]
                    if hf == 0:
                        projs.append((wq, bq_sb, qt_d, False))
                    for w_in, b_sb, out_d, is_k in projs:
                        for m in range(DK):
                            wm = p1w.tile([128, DK * 128], F32R, tag="wm")
                            nc.sync.dma_start(
                                out=wm[:],
                                in_=w_in.rearrange("(k p) n -> p k n", p=128)[
                                    :, :, m * 128:(m + 1) * 128
                                ],
                            )
                            for jt in range(T2 // NCH):
                                ps = ps_kq.tile([128, NCH], F32, tag="pskq")
                                for k in range(DK):
                                    nc.tensor.matmul(
                                        ps[:],
                                        _r(wm[:, k * 128:(k + 1) * 128]),
                                        _r(slab[:, k * T2 + jt * NCH:
                                                k * T2 + (jt + 1) * NCH]),
                                        start=(k == 0),
                                        stop=(k == DK - 1),
                                    )
                                st = p1st.tile([128, NCH], F32R, tag="kqst")
                                nc.vector.tensor_scalar_add(
                                    st[:], ps[:], b_sb[:, m:m + 1]
                                )
                                col0 = (hf * T2 if is_k else 0) + jt * NCH
                                nc.sync.dma_start(
                                    out=out_d[m * 128:(m + 1) * 128,
                                              col0:col0 + NCH],
                                    in_=st[:],
                                )
                    # V projection (natural layout), n-chunks of 512
                    for nb_ in range(D // min(512, D)):
                        NV = min(512, D)
                        wvn = p1wv.tile([128, DK * NV], F32R, tag="wvn")
                        nc.sync.dma_start(
                            out=wvn[:],
                            in_=wv.rearrange("(k p) n -> p k n", p=128)[
                                :, :, nb_ * NV:(nb_ + 1) * NV
                            ],
                        )
                        for tcn in range(T2 // 128):
                            ps = ps_v.tile([128, NV], F32, tag="psv")
                            for k in range(DK):
                                nc.tensor.matmul(
                                    ps[:],
                                    _r(slab[:, k * T2 + tcn * 128:
                                            k * T2 + (tcn + 1) * 128]),
                                    _r(wvn[:, k * NV:(k + 1) * NV]),
                                    start=(k == 0),
                                    stop=False,
                                )
                            nc.tensor.matmul(
                                ps[:],
                                _r(ones_row[:]),
                                _r(bv_sb[:, nb_ * NV:(nb_ + 1) * NV]),
                                start=False,
                                stop=True,
                            )
                            st = p1st.tile([128, NV], F32R, tag="vst")
                            nc.scalar.copy(st[:], ps[:])
                            nc.sync.dma_start(
                                out=v_d[hf * T2 + tcn * 128:
                                        hf * T2 + (tcn + 1) * 128,
                                        nb_ * NV:(nb_ + 1) * NV],
                                in_=st[:],
                            )

            # ---------------- phase 2+3: attention per head ----------------
            with tc.tile_pool(name="aslab", bufs=1) as paslab:
              at_sb = paslab.tile([128, H * OWN], F32R, tag="aslab")
              with (
                tc.tile_pool(name="pmask", bufs=1) as pmask,
                tc.tile_pool(name="ph", bufs=2) as ph,
                tc.tile_pool(name="ppt", bufs=3) as ppt,
                tc.tile_pool(name="psm", bufs=2) as psm,
                tc.tile_pool(name="ps_s", bufs=2, space="PSUM") as ps_s,
                tc.tile_pool(name="ps_a", bufs=1, space="PSUM") as ps_a,
                tc.tile_pool(name="ps_l", bufs=1, space="PSUM") as ps_l,
            ):
                mask_sb = pmask.tile([128, 4 * q * QW], F32R, tag="mask")
                nc.sync.dma_start(out=mask_sb[:], in_=mask[:])
                NS = min(512, OWN)
                for h in range(H):
                    kt_h = ph.tile([128, T], F32R, tag="kth")
                    nc.sync.dma_start(
                        out=kt_h[:], in_=kt_d[h * 128:(h + 1) * 128, :]
                    )
                    qt_h = ph.tile([128, OWN], F32R, tag="qth")
                    nc.sync.dma_start(
                        out=qt_h[:], in_=qt_d[h * 128:(h + 1) * 128, :]
                    )
                    v_h = ph.tile([128, T], F32R, tag="vh")
                    nc.sync.dma_start(
                        out=v_h[:].rearrange("p (jb c) -> p jb c", c=128),
                        in_=v_d.rearrange("(jb p) d -> p jb d", p=128)[
                            :, :, h * 128:(h + 1) * 128
                        ],
                    )
                    psa = ps_a.tile([128, OWN], F32, tag="psa")
                    psl = ps_l.tile([1, OWN], F32, tag="psl")
                    h_own_bank = QW * 4 >= 2048
                    n_tiles = len(LWIN) + len(HONLY)

                    def consume(jb, ptv, full, pos):
                        # AV + denominator matmuls for a tile whose exp/mask
                        # chain was issued one pipeline step earlier.
                        vt = _r(v_h[:, jb * 128:(jb + 1) * 128])
                        first = pos == 0
                        # stop clears the (bank-granular) sim group flag, so in
                        # the shared-bank layout only the final H write stops
                        last_l = (pos == len(LWIN) - 1) and h_own_bank
                        last_h = pos == n_tiles - 1
                        if first and not h_own_bank:
                            nc.tensor.matmul(
                                psa[:, :OWN], vt, _r(ptv[:, :OWN]),
                                start=True, stop=False,
                            )
                            nc.tensor.matmul(
                                psl[:, :OWN], _r(ones_col[:]), _r(ptv[:, :OWN]),
                                start=True, stop=False,
                            )
                            return
                        if full:
                            nc.tensor.matmul(
                                psa[:, :QW], vt, _r(ptv[:, :QW]),
                                start=first, stop=last_l,
                            )
                            nc.tensor.matmul(
                                psa[:, QW:OWN], vt, _r(ptv[:, QW:OWN]),
                                start=first and h_own_bank, stop=last_h,
                            )
                            nc.tensor.matmul(
                                psl[:, :QW], _r(ones_col[:]), _r(ptv[:, :QW]),
                                start=first, stop=last_l,
                            )
                            nc.tensor.matmul(
                                psl[:, QW:OWN], _r(ones_col[:]),
                                _r(ptv[:, QW:OWN]),
                                start=first and h_own_bank, stop=last_h,
                            )
                        else:
                            nc.tensor.matmul(
                                psa[:, QW:OWN], vt, _r(ptv[:, :QW]),
                                start=False, stop=last_h,
                            )
                            nc.tensor.matmul(
                                psl[:, QW:OWN], _r(ones_col[:]),
                                _r(ptv[:, :QW]),
                                start=False, stop=last_h,
                            )

                    # units: full tiles singly; H-only tiles in PAIRS sharing
                    # one PSUM slot, one wide exp and one wide mask-multiply
                    # (halves ACT/DVE instruction overhead in the softmax).
                    units = [("full", (jb,)) for jb in LWIN] + [
                        ("hpair", tuple(HONLY[i:i + 2]))
                        for i in range(0, len(HONLY), 2)
                    ]
                    pos = 0
                    pending = []
                    for kind, jjs in units:
                        pss = ps_s.tile([128, OWN], F32, tag="pss")
                        pt = ppt.tile([128, OWN], F32R, tag="pt")
                        if kind == "full":
                            (jb,) = jjs
                            ns = min(NS, OWN)
                            for sc in range(OWN // ns):
                                nc.tensor.matmul(
                                    pss[:, sc * ns:(sc + 1) * ns],
                                    _r(kt_h[:, jb * 128:(jb + 1) * 128]),
                                    _r(qt_h[:, sc * ns:(sc + 1) * ns]),
                                    start=True, stop=True,
                                )
                            nc.scalar.activation(pt[:], pss[:], EXP, scale=SCALE)
                            mc = LWIN.index(jb) * QW
                            nc.vector.tensor_mul(
                                pt[:, :QW], pt[:, :QW], mask_sb[:, mc:mc + QW]
                            )
                            fresh = [(jb, pt[:], True)]
                        else:
                            for half, jb in enumerate(jjs):
                                nc.tensor.matmul(
                                    pss[:, half * QW:(half + 1) * QW],
                                    _r(kt_h[:, jb * 128:(jb + 1) * 128]),
                                    _r(qt_h[:, QW:OWN]),
                                    start=True, stop=True,
                                )
                            w = len(jjs) * QW
                            nc.scalar.activation(
                                pt[:, :w], pss[:, :w], EXP, scale=SCALE
                            )
                            mc = (2 * q + HONLY.index(jjs[0])) * QW
                            nc.vector.tensor_mul(
                                pt[:, :w], pt[:, :w], mask_sb[:, mc:mc + w]
                            )
                            fresh = [
                                (jb, pt[:, half * QW:(half + 1) * QW], False)
                                for half, jb in enumerate(jjs)
                            ]
                        if not PIPELINE:
                            pending.extend(fresh)
                            fresh = []
                        for jb_, ptv_, full_ in pending:
                            consume(jb_, ptv_, full_, pos)
                            pos += 1
                        pending = fresh
                    for jb_, ptv_, full_ in pending:
                        consume(jb_, ptv_, full_, pos)
                        pos += 1
                    # Evacuate both PSUM accumulators with fast ACT copies so
                    # the next head's matmuls aren't gated on the (slow)
                    # reciprocal / broadcast / normalize chain below.
                    l_raw = psm.tile([1, OWN], F32, tag="lraw")
                    nc.vector.tensor_copy(l_raw[:], psl[:])
                    at_raw = psm.tile([128, OWN], F32, tag="atraw")
                    nc.vector.tensor_copy(at_raw[:], psa[:])
                    l_sb = psm.tile([1, OWN], F32, tag="lsb")
                    nc.vector.reciprocal_approx_fast(l_sb[:], l_raw[:])
                    lb = psm.tile([128, OWN], F32, tag="lb")
                    nc.gpsimd.partition_broadcast(lb[:], l_sb[:], channels=128)
                    nc.vector.tensor_tensor(
                        at_sb[:, h * OWN:(h + 1) * OWN], at_raw[:], lb[:], MULT
                    )

              # ---------------- phase 4: output projection ----------------
              with (
                  tc.tile_pool(name="p4w", bufs=2) as p4w,
                  tc.tile_pool(name="p4st", bufs=2) as p4st,
                  tc.tile_pool(name="ps_o", bufs=2, space="PSUM") as ps_o,
              ):
                  for nh in range(D // ND):
                      won = p4w.tile([128, DK * ND], F32R, tag="won")
                      nc.sync.dma_start(
                          out=won[:],
                          in_=wo.rearrange("(k p) n -> p k n", p=128)[
                              :, :, nh * ND:(nh + 1) * ND
                          ],
                      )
                      for tt in range(OWN // 128):
                          pso = ps_o.tile([128, ND], F32, tag="pso")
                          for k in range(DK):
                              for sc in range(ND // min(512, ND)):
                                  NO = min(512, ND)
                                  nc.tensor.matmul(
                                      pso[:, sc * NO:(sc + 1) * NO],
                                      at_sb[:, k * OWN + tt * 128:
                                            k * OWN + (tt + 1) * 128],
                                      _r(won[:, k * ND + sc * NO:
                                             k * ND + (sc + 1) * NO]),
                                      start=(k == 0),
                                      stop=False,
                                  )
                          for sc in range(ND // min(512, ND)):
                              NO = min(512, ND)
                              nc.tensor.matmul(
                                  pso[:, sc * NO:(sc + 1) * NO],
                                  _r(ones_row[:]),
                                  _r(bo_sb[:, nh * ND + sc * NO:
                                           nh * ND + (sc + 1) * NO]),
                                  start=False,
                                  stop=True,
                              )
                          ost = p4st.tile([128, ND], F32, tag="ost")
                          nc.scalar.copy(ost[:], pso[:])
                          nc.sync.dma_start(
                              out=o[tt * 128:(tt + 1) * 128, nh * ND:(nh + 1) * ND],
                              in_=ost[:],
                          )
    nc.compile()
    return nc


def host_shard(cfg, x_full):
    """Per-core permutations, permuted x, and mask tensors.

    Returns (perms, x_ins, masks): lists indexed by core = 2*b + z.
    """
    c = _derived(cfg)
    B, T, QW, OWN, q = c["B"], c["T"], c["QW"], c["OWN"], c["q"]
    quarters = [np.arange(i * QW, (i + 1) * QW) for i in range(4)]
    LWIN = list(range(q)) + list(range(2 * q, 3 * q))
    HONLY = list(range(q, 2 * q)) + list(range(3 * q, 4 * q))
    perms, x_ins, masks = [], [], []
    for b in range(B):
        for z in range(2):
            if z == 0:
                own = [quarters[0], quarters[3]]
                rest = [quarters[1], quarters[2]]
            else:
                own = [quarters[1], quarters[2]]
                rest = [quarters[0], quarters[3]]
            perm = np.concatenate(own + rest)
            perms.append(perm)
            x_ins.append(np.ascontiguousarray(x_full[b][perm]))
            m = np.empty((128, 4 * q * QW), dtype=np.float32)
            ig_L = perm[:QW]
            ig_H = perm[QW:OWN]
            for t, jb in enumerate(LWIN):
                jg = perm[jb * 128:(jb + 1) * 128]
                m[:, t * QW:(t + 1) * QW] = (
                    jg[:, None] <= ig_L[None, :]
                ).astype(np.float32)
            for t, jb in enumerate(HONLY):
                jg = perm[jb * 128:(jb + 1) * 128]
                m[:, (2 * q + t) * QW:(2 * q + t + 1) * QW] = (
                    jg[:, None] <= ig_H[None, :]
                ).astype(np.float32)
            masks.append(m)
    return perms, x_ins, masks


def run_cores(cfg, nc, inputs, perms, x_ins, masks, trace=False, tmpdir=None):
    c = _derived(cfg)
    n = c["N_CORES"]
    f32 = np.float32
    shared = {
        "wq": np.ascontiguousarray(inputs["Wq"], f32),
        "wk": np.ascontiguousarray(inputs["Wk"], f32),
        "wv": np.ascontiguousarray(inputs["Wv"], f32),
        "wo": np.ascontiguousarray(inputs["Wo"], f32),
        "bq": np.ascontiguousarray(inputs["bq"], f32),
        "bk": np.ascontiguousarray(inputs["bk"], f32),
        "bv": np.ascontiguousarray(inputs["bv"], f32),
        "bo": np.ascontiguousarray(inputs["bo"], f32),
    }
    consts = {
        "ident": np.eye(128, dtype=f32),
        "ones_c": np.ones((128, 1), f32),
        "ones_r": np.ones((1, 128), f32),
    }
    in_maps = [
        {"x": x_ins[i], "mask": masks[i], **consts, **shared} for i in range(n)
    ]
    res = run_bass_kernel_spmd(
        nc, in_maps, list(range(n)), trace=trace, tmpdir=tmpdir
    )
    B, T, D, OWN = c["B"], c["T"], c["D"], c["OWN"]
    out = np.empty((B, T, D), dtype=np.float32)
    for b in range(B):
        for z in range(2):
            core = 2 * b + z
            out[b][perms[core][:OWN]] = res.results[core]["o"]
    return out, res


_NC_CACHE = {}


def kernel(x, Wq, bq, Wk, bk, Wv, bv, Wo, bo):
    cfg = PROD_CFG
    key = tuple(sorted(cfg.items()))
    if key not in _NC_CACHE:
        _NC_CACHE[key] = build_nc(cfg)
    nc = _NC_CACHE[key]
    x = np.asarray(x, np.float32)
    perms, x_ins, masks = host_shard(cfg, x)
    inputs = dict(Wq=Wq, bq=bq, Wk=Wk, bk=bk, Wv=Wv, bv=bv, Wo=Wo, bo=bo)
    out, _ = run_cores(cfg, nc, inputs, perms, x_ins, masks)
    return out



# revision 5
# speedup vs baseline: 1.8558x; 1.8558x over previous
"""Causal multi-head attention (B=4, T=2048, D=2048, H=16) on 8 Trainium2
NeuronCores via Bass/Tile, SPMD with zero collectives.

Sharding: head-split tensor parallelism. Core pair (2b, 2b+1) handles batch
b; core 2b computes heads 0-7, core 2b+1 heads 8-15 (identical instruction
streams -- the head split is just a different weight slice). Each core:
  - projects Q^T/K^T (own heads, all T positions) and V (own heads) from a
    host-pre-transposed, bf16-cast x^T,
  - runs the full causal triangle for its 8 heads with suffix-window score
    tiles (keys tile j attends to the contiguous query suffix [128j, T)),
    only diagonal 128x128 blocks need a mask multiply,
  - emits a PARTIAL output projection o_part = A_own @ Wo[own rows, :].
The host sums the two partials of each pair and adds bo during unshard.

Numerics: all matmul operands bf16 (hosts pre-casts x/W; on-chip
intermediates are cast to bf16 during PSUM evacuation), f32 PSUM
accumulation throughout; exp without max-subtraction (scores are O(1));
bk dropped (softmax-invariant); bv folded into the post-softmax normalize.
Max rel err vs f32 reference ~3.5e-3 (numpy bit-exact simulation).
"""
import numpy as np
import ml_dtypes

import concourse.bacc as bacc
import concourse.mybir as mybir
from concourse.tile import TileContext
from concourse.bass_utils import run_bass_kernel_spmd

F32 = mybir.dt.float32
BF16 = mybir.dt.bfloat16
EXP = mybir.ActivationFunctionType.Exp
MULT = mybir.AluOpType.mult

PROD_CFG = dict(B=4, T=2048, D=2048, H=16)


def _derived(cfg):
    B, T, D, H = cfg["B"], cfg["T"], cfg["D"], cfg["H"]
    d = dict(cfg)
    d.update(
        DH=128,
        HH=H // 2,             # own heads per core
        DO=D // 2,             # own output-dim slice (HH * DH)
        DK=D // 128,           # contraction chunks over D
        QH=T // 2,             # query-half width for PSUM blocking
        N_CORES=2 * B,
    )
    return d


def _qh_units(c, qh):
    """Schedule for one query half: list of (jb, c0) key-tile units.

    Unit (jb, c0): key tile jb attends query cols [c0, QH) of this half,
    diagonal (mask) iff the suffix starts at the tile's own query block.
    """
    QH = c["QH"]
    nt = QH // 128
    if qh == 0:
        return [(j, 128 * j, True) for j in range(nt)]
    full = [(j, 0, False) for j in range(nt)]
    diag = [(nt + j, 128 * j, True) for j in range(nt)]
    return full + diag


def build_nc(cfg):
    c = _derived(cfg)
    T, D = c["T"], c["D"]
    HH, DO, DK, QH = c["HH"], c["DO"], c["DK"], c["QH"]
    SCALE = float(c["DH"] ** -0.5)

    nc = bacc.Bacc(
        "TRN2", target_bir_lowering=False, debug=False, num_devices=c["N_CORES"]
    )
    xt = nc.dram_tensor("xt", [D, T], BF16, kind="ExternalInput").ap()
    wq = nc.dram_tensor("wq", [D, DO], BF16, kind="ExternalInput").ap()
    wk = nc.dram_tensor("wk", [D, DO], BF16, kind="ExternalInput").ap()
    wv = nc.dram_tensor("wv", [D, DO], BF16, kind="ExternalInput").ap()
    wo = nc.dram_tensor("wo", [DO, D], BF16, kind="ExternalInput").ap()
    bq = nc.dram_tensor("bq", [DO], F32, kind="ExternalInput").ap()
    bv = nc.dram_tensor("bv", [DO], F32, kind="ExternalInput").ap()
    mask_in = nc.dram_tensor("mask", [128, 128], BF16, kind="ExternalInput").ap()
    ones_in = nc.dram_tensor("ones_c", [128, 1], BF16, kind="ExternalInput").ap()
    o = nc.dram_tensor("o", [T, D], F32, kind="ExternalOutput").ap()

    with TileContext(nc) as tc:
        with (
            tc.tile_pool(name="const", bufs=1) as pconst,
            tc.tile_pool(name="kqv", bufs=1) as pkqv,
        ):
            mask_sb = pconst.tile([128, 128], BF16, tag="mask")
            nc.sync.dma_start(out=mask_sb[:], in_=mask_in[:])
            ones_col = pconst.tile([128, 1], BF16, tag="ones_col")
            nc.sync.dma_start(out=ones_col[:], in_=ones_in[:])
            bq_sb = pconst.tile([128, HH], F32, tag="bq")
            nc.sync.dma_start(out=bq_sb[:], in_=bq.rearrange("(m p) -> p m", p=128))
            bv_sb = pconst.tile([128, HH], F32, tag="bv")
            nc.sync.dma_start(out=bv_sb[:], in_=bv.rearrange("(m p) -> p m", p=128))

            kt_sb = pkqv.tile([128, HH, T], BF16, tag="kt")
            qt_sb = pkqv.tile([128, HH, T], BF16, tag="qt")
            v_sb = pkqv.tile([128, T // 128, DO], BF16, tag="v")

            # ---------------- phase B: Q^T, K^T, V projections ----------------
            with (
                tc.tile_pool(name="pxt", bufs=1) as pxt,
                tc.tile_pool(name="pw", bufs=2) as pw,
                tc.tile_pool(name="ps_p", bufs=3, space="PSUM") as ps_p,
            ):
                xt_sb = pxt.tile([128, DK, T], BF16, tag="xt")
                for k in range(DK):
                    nc.sync.dma_start(
                        out=xt_sb[:, k, :],
                        in_=xt.rearrange("(k p) t -> p k t", p=128)[:, k, :],
                    )
                # Q^T / K^T: out[dh_p, t], stationary = weight chunk
                for w_in, is_q in ((wq, True), (wk, False)):
                    for m in range(HH):
                        wm = pw.tile([128, DK, 128], BF16, tag="wm")
                        nc.sync.dma_start(
                            out=wm[:],
                            in_=w_in.rearrange("(k p) n -> p k n", p=128)[
                                :, :, m * 128:(m + 1) * 128
                            ],
                        )
                        for tcn in range(T // 512):
                            ps = ps_p.tile([128, 512], F32, tag="psp")
                            for k in range(DK):
                                nc.tensor.matmul(
                                    ps[:],
                                    wm[:, k, :],
                                    xt_sb[:, k, tcn * 512:(tcn + 1) * 512],
                                    start=(k == 0),
                                    stop=(k == DK - 1),
                                )
                            if is_q:
                                nc.vector.tensor_scalar_add(
                                    qt_sb[:, m, tcn * 512:(tcn + 1) * 512],
                                    ps[:], bq_sb[:, m:m + 1],
                                )
                            else:
                                nc.scalar.copy(
                                    kt_sb[:, m, tcn * 512:(tcn + 1) * 512], ps[:]
                                )
                # V: out[t_p, n], stationary = xt chunk, moving = wv
                for nb in range(DO // 512):
                    wvn = pw.tile([128, DK, 512], BF16, tag="wvn")
                    nc.sync.dma_start(
                        out=wvn[:],
                        in_=wv.rearrange("(k p) n -> p k n", p=128)[
                            :, :, nb * 512:(nb + 1) * 512
                        ],
                    )
                    for tt in range(T // 128):
                        ps = ps_p.tile([128, 512], F32, tag="psp")
                        for k in range(DK):
                            nc.tensor.matmul(
                                ps[:],
                                xt_sb[:, k, tt * 128:(tt + 1) * 128],
                                wvn[:, k, :],
                                start=(k == 0),
                                stop=(k == DK - 1),
                            )
                        nc.vector.tensor_copy(
                            v_sb[:, tt, nb * 512:(nb + 1) * 512], ps[:]
                        )

            # ---------------- phase C: attention per head ----------------
            with tc.tile_pool(name="pat", bufs=1) as pat:
              at_sb = pat.tile([128, HH, T], BF16, tag="at")
              with (
                  tc.tile_pool(name="ppt", bufs=3) as ppt,
                  tc.tile_pool(name="psm", bufs=2) as psm,
                  tc.tile_pool(name="ps_s", bufs=2, space="PSUM") as ps_s,
                  tc.tile_pool(name="ps_a", bufs=1, space="PSUM") as ps_a,
                  tc.tile_pool(name="ps_l", bufs=1, space="PSUM") as ps_l,
              ):
                for h in range(HH):
                    for qh in range(2):
                        q0 = qh * QH
                        units = _qh_units(c, qh)
                        psa = ps_a.tile([128, QH], F32, tag="psa")
                        psl = ps_l.tile([1, QH], F32, tag="psl")
                        # last unit touching cols [0,512) per accumulator
                        last_b0 = max(i for i, u in enumerate(units) if u[1] < 512)
                        n_units = len(units)

                        def consume(pos, jb, c0, pt_ap):
                            # PSUM regions must not cross 512-col bank edges:
                            # chunk A [c0, 512), chunk B [512, QH)
                            vt = v_sb[:, jb, h * 128:(h + 1) * 128]
                            chunks = []
                            if c0 < 512:
                                chunks.append((c0, 512 - c0, pos == last_b0))
                                chunks.append((512, QH - 512, pos == n_units - 1))
                            else:
                                chunks.append((c0, QH - c0, pos == n_units - 1))
                            for cs, cw, is_stop in chunks:
                                nc.tensor.matmul(
                                    psa[:, cs:cs + cw],
                                    vt,
                                    pt_ap[:, cs:cs + cw],
                                    start=(pos == 0),
                                    stop=is_stop,
                                )
                                nc.tensor.matmul(
                                    psl[:, cs:cs + cw],
                                    ones_col[:],
                                    pt_ap[:, cs:cs + cw],
                                    start=(pos == 0),
                                    stop=is_stop,
                                )

                        pending = None
                        for pos, (jb, c0, diag) in enumerate(units):
                            pss = ps_s.tile([128, QH], F32, tag="pss")
                            pt = ppt.tile([128, QH], BF16, tag="pt")
                            s_chunks = (
                                [(c0, 512 - c0), (512, QH - 512)]
                                if c0 < 512 else [(c0, QH - c0)]
                            )
                            for cs, cw in s_chunks:
                                nc.tensor.matmul(
                                    pss[:, cs:cs + cw],
                                    kt_sb[:, h, jb * 128:(jb + 1) * 128],
                                    qt_sb[:, h, q0 + cs:q0 + cs + cw],
                                    start=True, stop=True,
                                )
                            nc.scalar.activation(
                                pt[:, c0:QH], pss[:, c0:QH], EXP, scale=SCALE
                            )
                            if diag:
                                nc.vector.tensor_mul(
                                    pt[:, c0:c0 + 128], pt[:, c0:c0 + 128],
                                    mask_sb[:],
                                )
                            if pending is not None:
                                consume(*pending)
                            pending = (pos, jb, c0, pt[:])
                        consume(*pending)
                        # evacuate + normalize (off PE critical path)
                        l_raw = psm.tile([1, QH], F32, tag="lraw")
                        nc.vector.tensor_copy(l_raw[:], psl[:])
                        at_raw = psm.tile([128, QH], F32, tag="atraw")
                        nc.scalar.copy(at_raw[:], psa[:])
                        l_inv = psm.tile([1, QH], F32, tag="linv")
                        nc.vector.reciprocal_approx_fast(l_inv[:], l_raw[:])
                        lb = psm.tile([128, QH], F32, tag="lb")
                        nc.gpsimd.partition_broadcast(lb[:], l_inv[:], channels=128)
                        at_tmp = psm.tile([128, QH], F32, tag="attmp")
                        nc.vector.tensor_tensor(at_tmp[:], at_raw[:], lb[:], MULT)
                        nc.vector.tensor_scalar_add(
                            at_sb[:, h, q0:q0 + QH], at_tmp[:], bv_sb[:, h:h + 1]
                        )

              # ---------------- phase D: partial output projection ----------------
              with (
                  tc.tile_pool(name="pwo", bufs=1) as pwo,
                  tc.tile_pool(name="post", bufs=3) as post,
                  tc.tile_pool(name="ps_o", bufs=3, space="PSUM") as ps_o,
              ):
                  wo_sb = pwo.tile([128, HH, D], BF16, tag="wo")
                  for k in range(HH):
                      nc.sync.dma_start(
                          out=wo_sb[:, k, :],
                          in_=wo.rearrange("(k p) n -> p k n", p=128)[:, k, :],
                      )
                  for tt in range(T // 128):
                      for cc in range(D // 512):
                          pso = ps_o.tile([128, 512], F32, tag="pso")
                          for k in range(HH):
                              nc.tensor.matmul(
                                  pso[:],
                                  at_sb[:, k, tt * 128:(tt + 1) * 128],
                                  wo_sb[:, k, cc * 512:(cc + 1) * 512],
                                  start=(k == 0),
                                  stop=(k == HH - 1),
                              )
                          ost = post.tile([128, 512], F32, tag="ost")
                          nc.scalar.copy(ost[:], pso[:])
                          nc.sync.dma_start(
                              out=o[tt * 128:(tt + 1) * 128,
                                    cc * 512:(cc + 1) * 512],
                              in_=ost[:],
                          )
    nc.compile()
    return nc


def host_shard(cfg, x_full, inputs):
    """Per-core input maps (head-split TP: core 2b+z = batch b, heads z*8..)."""
    c = _derived(cfg)
    B, DO = c["B"], c["DO"]
    bf = ml_dtypes.bfloat16
    f32 = np.float32
    mask = np.triu(np.ones((128, 128), dtype=f32)).astype(bf)
    ones_c = np.ones((128, 1), f32).astype(bf)
    wq, wk, wv, wo = (np.asarray(inputs[k], f32) for k in ["Wq", "Wk", "Wv", "Wo"])
    bq, bv = (np.asarray(inputs[k], f32) for k in ["bq", "bv"])
    in_maps = []
    for b in range(B):
        xtb = np.ascontiguousarray(np.asarray(x_full[b], f32).T).astype(bf)
        for z in range(2):
            sl = slice(z * DO, (z + 1) * DO)
            in_maps.append({
                "xt": xtb,
                "wq": np.ascontiguousarray(wq[:, sl]).astype(bf),
                "wk": np.ascontiguousarray(wk[:, sl]).astype(bf),
                "wv": np.ascontiguousarray(wv[:, sl]).astype(bf),
                "wo": np.ascontiguousarray(wo[sl, :]).astype(bf),
                "bq": np.ascontiguousarray(bq[sl]),
                "bv": np.ascontiguousarray(bv[sl]),
                "mask": mask,
                "ones_c": ones_c,
            })
    return in_maps


def run_cores(cfg, nc, in_maps, bo, trace=False, tmpdir=None):
    c = _derived(cfg)
    n = c["N_CORES"]
    res = run_bass_kernel_spmd(
        nc, in_maps, list(range(n)), trace=trace, tmpdir=tmpdir
    )
    B, T, D = c["B"], c["T"], c["D"]
    out = np.empty((B, T, D), dtype=np.float32)
    bo = np.asarray(bo, np.float32)
    for b in range(B):
        out[b] = res.results[2 * b]["o"] + res.results[2 * b + 1]["o"] + bo
    return out, res


_NC_CACHE = {}


def kernel(x, Wq, bq, Wk, bk, Wv, bv, Wo, bo):
    cfg = PROD_CFG
    key = tuple(sorted(cfg.items()))
    if key not in _NC_CACHE:
        _NC_CACHE[key] = build_nc(cfg)
    nc = _NC_CACHE[key]
    inputs = dict(Wq=Wq, bq=bq, Wk=Wk, bk=bk, Wv=Wv, bv=bv, Wo=Wo, bo=bo)
    in_maps = host_shard(cfg, np.asarray(x, np.float32), inputs)
    out, _ = run_cores(cfg, nc, in_maps, bo)
    return out


# revision 9
# speedup vs baseline: 1.9161x; 1.0325x over previous
"""Causal multi-head attention (B=4, T=2048, D=2048, H=16) on 8 Trainium2
NeuronCores via Bass/Tile, SPMD with zero collectives.

Sharding: head-split tensor parallelism. Core pair (2b, 2b+1) handles batch
b; core 2b computes heads 0-7, core 2b+1 heads 8-15 (identical instruction
streams -- the head split is just a different weight slice). Each core:
  - projects Q^T/K^T (own heads, all T positions) and V (own heads) from a
    host-pre-transposed, bf16-cast x^T,
  - runs the full causal triangle for its 8 heads with suffix-window score
    tiles (keys tile j attends to the contiguous query suffix [128j, T)),
    only diagonal 128x128 blocks need a mask multiply,
  - emits a PARTIAL output projection o_part = A_own @ Wo[own rows, :].
The host sums the two partials of each pair and adds bo during unshard.

Numerics: all matmul operands bf16 (hosts pre-casts x/W; on-chip
intermediates are cast to bf16 during PSUM evacuation), f32 PSUM
accumulation throughout; exp without max-subtraction (scores are O(1));
bk dropped (softmax-invariant); bv folded into the post-softmax normalize.
Max rel err vs f32 reference ~3.5e-3 (numpy bit-exact simulation).
"""
import numpy as np
import ml_dtypes

import concourse.bacc as bacc
import concourse.mybir as mybir
from concourse.tile import TileContext
from concourse.bass_utils import run_bass_kernel_spmd

F32 = mybir.dt.float32
BF16 = mybir.dt.bfloat16
EXP = mybir.ActivationFunctionType.Exp
MULT = mybir.AluOpType.mult

PROD_CFG = dict(B=4, T=2048, D=2048, H=16)


def _derived(cfg):
    B, T, D, H = cfg["B"], cfg["T"], cfg["D"], cfg["H"]
    d = dict(cfg)
    d.update(
        DH=128,
        HH=H // 2,             # own heads per core
        DO=D // 2,             # own output-dim slice (HH * DH)
        DK=D // 128,           # contraction chunks over D
        QH=T // 2,             # query-half width for PSUM blocking
        N_CORES=2 * B,
    )
    return d


def _qq_units(qq):
    """Schedule for query quarter qq (512 cols): list of (jb, c0, diag).

    Key tile jb attends query cols [c0, 512) of the quarter; the 4 tiles
    at the causal diagonal get a mask multiply on their first 128 cols.
    """
    full = [(j, 0, False) for j in range(4 * qq)]
    diag = [(4 * qq + j, 128 * j, True) for j in range(4)]
    return full + diag


def build_nc(cfg):
    c = _derived(cfg)
    T, D = c["T"], c["D"]
    HH, DO, DK, QH = c["HH"], c["DO"], c["DK"], c["QH"]
    SCALE = float(c["DH"] ** -0.5)

    nc = bacc.Bacc(
        "TRN2", target_bir_lowering=False, debug=False, num_devices=c["N_CORES"]
    )
    xt = nc.dram_tensor("xt", [D, T], BF16, kind="ExternalInput").ap()
    wq = nc.dram_tensor("wq", [D, DO], BF16, kind="ExternalInput").ap()
    wk = nc.dram_tensor("wk", [D, DO], BF16, kind="ExternalInput").ap()
    wv = nc.dram_tensor("wv", [D, DO], BF16, kind="ExternalInput").ap()
    wo = nc.dram_tensor("wo", [DO, D], BF16, kind="ExternalInput").ap()
    bq = nc.dram_tensor("bq", [DO], F32, kind="ExternalInput").ap()
    bv = nc.dram_tensor("bv", [DO], F32, kind="ExternalInput").ap()
    mask_in = nc.dram_tensor("mask", [128, 128], BF16, kind="ExternalInput").ap()
    ones_in = nc.dram_tensor("ones_c", [128, 1], BF16, kind="ExternalInput").ap()
    o = nc.dram_tensor("o", [T, D], F32, kind="ExternalOutput").ap()

    with TileContext(nc) as tc:
        with (
            tc.tile_pool(name="const", bufs=1) as pconst,
            tc.tile_pool(name="kqv", bufs=1) as pkqv,
        ):
            mask_sb = pconst.tile([128, 128], BF16, tag="mask")
            nc.sync.dma_start(out=mask_sb[:], in_=mask_in[:])
            ones_col = pconst.tile([128, 1], BF16, tag="ones_col")
            nc.sync.dma_start(out=ones_col[:], in_=ones_in[:])
            bq_sb = pconst.tile([128, HH], F32, tag="bq")
            nc.sync.dma_start(out=bq_sb[:], in_=bq.rearrange("(m p) -> p m", p=128))
            bv_sb = pconst.tile([128, HH], F32, tag="bv")
            nc.sync.dma_start(out=bv_sb[:], in_=bv.rearrange("(m p) -> p m", p=128))

            kt_sb = pkqv.tile([128, HH, T], BF16, tag="kt")
            qt_sb = pkqv.tile([128, HH, T], BF16, tag="qt")
            v_sb = pkqv.tile([128, T // 128, DO], BF16, tag="v")

            # ---------------- phase B: Q^T, K^T, V projections ----------------
            with (
                tc.tile_pool(name="pxt", bufs=1) as pxt,
                tc.tile_pool(name="pw", bufs=2) as pw,
                tc.tile_pool(name="ps_p", bufs=3, space="PSUM") as ps_p,
            ):
                # x^T on the scalar queue so weight DMAs (sync queue) are
                # not stuck behind the 8MB load; k-chunk matmuls start as
                # soon as their chunk lands.
                xt_sb = pxt.tile([128, DK, T], BF16, tag="xt")
                for k in range(DK):
                    nc.scalar.dma_start(
                        out=xt_sb[:, k, :],
                        in_=xt.rearrange("(k p) t -> p k t", p=128)[:, k, :],
                    )
                # Q^T / K^T: out[dh_p, t], stationary = weight chunk
                for w_in, is_q in ((wq, True), (wk, False)):
                    for m in range(HH):
                        wm = pw.tile([128, DK, 128], BF16, tag="wm")
                        nc.sync.dma_start(
                            out=wm[:],
                            in_=w_in.rearrange("(k p) n -> p k n", p=128)[
                                :, :, m * 128:(m + 1) * 128
                            ],
                        )
                        for tcn in range(T // 512):
                            ps = ps_p.tile([128, 512], F32, tag="psp")
                            for k in range(DK):
                                nc.tensor.matmul(
                                    ps[:],
                                    wm[:, k, :],
                                    xt_sb[:, k, tcn * 512:(tcn + 1) * 512],
                                    start=(k == 0),
                                    stop=(k == DK - 1),
                                )
                            if is_q:
                                nc.vector.tensor_scalar_add(
                                    qt_sb[:, m, tcn * 512:(tcn + 1) * 512],
                                    ps[:], bq_sb[:, m:m + 1],
                                )
                            else:
                                nc.scalar.copy(
                                    kt_sb[:, m, tcn * 512:(tcn + 1) * 512], ps[:]
                                )
                # V: out[t_p, n], stationary = xt chunk, moving = wv
                for nb in range(DO // 512):
                    wvn = pw.tile([128, DK, 512], BF16, tag="wvn")
                    nc.sync.dma_start(
                        out=wvn[:],
                        in_=wv.rearrange("(k p) n -> p k n", p=128)[
                            :, :, nb * 512:(nb + 1) * 512
                        ],
                    )
                    for tt in range(T // 128):
                        ps = ps_p.tile([128, 512], F32, tag="psp")
                        for k in range(DK):
                            nc.tensor.matmul(
                                ps[:],
                                xt_sb[:, k, tt * 128:(tt + 1) * 128],
                                wvn[:, k, :],
                                start=(k == 0),
                                stop=(k == DK - 1),
                            )
                        nc.vector.tensor_copy(
                            v_sb[:, tt, nb * 512:(nb + 1) * 512], ps[:]
                        )

            # ---------------- phase C: attention per head ----------------
            with (
                tc.tile_pool(name="pat", bufs=1) as pat,
                tc.tile_pool(name="pwo", bufs=1) as pwo,
            ):
              at_sb = pat.tile([128, HH, T], BF16, tag="at")
              # prefetch Wo on an idle queue so phase D starts immediately
              wo_sb = pwo.tile([128, HH, D], BF16, tag="wo")
              for k in range(HH):
                  nc.gpsimd.dma_start(
                      out=wo_sb[:, k, :],
                      in_=wo.rearrange("(k p) n -> p k n", p=128)[:, k, :],
                  )
              with (
                  tc.tile_pool(name="ppt", bufs=4) as ppt,
                  tc.tile_pool(name="psm", bufs=2) as psm,
                  tc.tile_pool(name="ps_s", bufs=4, space="PSUM") as ps_s,
                  tc.tile_pool(name="ps_a", bufs=2, space="PSUM") as ps_a,
                  tc.tile_pool(name="ps_l", bufs=2, space="PSUM") as ps_l,
              ):
                QQ = 512
                for h in range(HH):
                    for qq in range(4):
                        q0 = qq * QQ
                        units = _qq_units(qq)
                        psa = ps_a.tile([128, QQ], F32, tag="psa")
                        psl = ps_l.tile([1, QQ], F32, tag="psl")
                        n_units = len(units)

                        def consume(pos, jb, c0, pt_ap):
                            vt = v_sb[:, jb, h * 128:(h + 1) * 128]
                            nc.tensor.matmul(
                                psa[:, c0:QQ],
                                vt,
                                pt_ap[:, c0:QQ],
                                start=(pos == 0),
                                stop=(pos == n_units - 1),
                            )
                            nc.tensor.matmul(
                                psl[:, c0:QQ],
                                ones_col[:],
                                pt_ap[:, c0:QQ],
                                start=(pos == 0),
                                stop=(pos == n_units - 1),
                            )

                        pending = None
                        for pos, (jb, c0, diag) in enumerate(units):
                            pss = ps_s.tile([128, QQ], F32, tag="pss")
                            pt = ppt.tile([128, QQ], BF16, tag="pt")
                            nc.tensor.matmul(
                                pss[:, c0:QQ],
                                kt_sb[:, h, jb * 128:(jb + 1) * 128],
                                qt_sb[:, h, q0 + c0:q0 + QQ],
                                start=True, stop=True,
                            )
                            nc.scalar.activation(
                                pt[:, c0:QQ], pss[:, c0:QQ], EXP, scale=SCALE
                            )
                            if diag:
                                nc.vector.tensor_mul(
                                    pt[:, c0:c0 + 128], pt[:, c0:c0 + 128],
                                    mask_sb[:],
                                )
                            if pending is not None:
                                consume(*pending)
                            pending = (pos, jb, c0, pt[:])
                        consume(*pending)
                        # evacuate + normalize (off PE critical path)
                        l_raw = psm.tile([1, QQ], F32, tag="lraw")
                        nc.vector.tensor_copy(l_raw[:], psl[:])
                        at_raw = psm.tile([128, QQ], F32, tag="atraw")
                        nc.scalar.copy(at_raw[:], psa[:])
                        l_inv = psm.tile([1, QQ], F32, tag="linv")
                        nc.vector.reciprocal_approx_fast(l_inv[:], l_raw[:])
                        lb = psm.tile([128, QQ], F32, tag="lb")
                        nc.gpsimd.partition_broadcast(lb[:], l_inv[:], channels=128)
                        at_tmp = psm.tile([128, QQ], F32, tag="attmp")
                        nc.vector.tensor_tensor(at_tmp[:], at_raw[:], lb[:], MULT)
                        nc.vector.tensor_scalar_add(
                            at_sb[:, h, q0:q0 + QQ], at_tmp[:], bv_sb[:, h:h + 1]
                        )

              # ---------------- phase D: partial output projection ----------------
              with (
                  tc.tile_pool(name="post", bufs=3) as post,
                  tc.tile_pool(name="ps_o", bufs=3, space="PSUM") as ps_o,
              ):
                  for tt in range(T // 128):
                      for cc in range(D // 512):
                          pso = ps_o.tile([128, 512], F32, tag="pso")
                          for k in range(HH):
                              nc.tensor.matmul(
                                  pso[:],
                                  at_sb[:, k, tt * 128:(tt + 1) * 128],
                                  wo_sb[:, k, cc * 512:(cc + 1) * 512],
                                  start=(k == 0),
                                  stop=(k == HH - 1),
                              )
                          ost = post.tile([128, 512], F32, tag="ost")
                          nc.scalar.copy(ost[:], pso[:])
                          nc.sync.dma_start(
                              out=o[tt * 128:(tt + 1) * 128,
                                    cc * 512:(cc + 1) * 512],
                              in_=ost[:],
                          )
    nc.compile()
    return nc


def host_shard(cfg, x_full, inputs):
    """Per-core input maps (head-split TP: core 2b+z = batch b, heads z*8..)."""
    c = _derived(cfg)
    B, DO = c["B"], c["DO"]
    bf = ml_dtypes.bfloat16
    f32 = np.float32
    mask = np.triu(np.ones((128, 128), dtype=f32)).astype(bf)
    ones_c = np.ones((128, 1), f32).astype(bf)
    wq, wk, wv, wo = (np.asarray(inputs[k], f32) for k in ["Wq", "Wk", "Wv", "Wo"])
    bq, bv = (np.asarray(inputs[k], f32) for k in ["bq", "bv"])
    in_maps = []
    for b in range(B):
        xtb = np.ascontiguousarray(np.asarray(x_full[b], f32).T).astype(bf)
        for z in range(2):
            sl = slice(z * DO, (z + 1) * DO)
            in_maps.append({
                "xt": xtb,
                "wq": np.ascontiguousarray(wq[:, sl]).astype(bf),
                "wk": np.ascontiguousarray(wk[:, sl]).astype(bf),
                "wv": np.ascontiguousarray(wv[:, sl]).astype(bf),
                "wo": np.ascontiguousarray(wo[sl, :]).astype(bf),
                "bq": np.ascontiguousarray(bq[sl]),
                "bv": np.ascontiguousarray(bv[sl]),
                "mask": mask,
                "ones_c": ones_c,
            })
    return in_maps


def run_cores(cfg, nc, in_maps, bo, trace=False, tmpdir=None):
    c = _derived(cfg)
    n = c["N_CORES"]
    res = run_bass_kernel_spmd(
        nc, in_maps, list(range(n)), trace=trace, tmpdir=tmpdir
    )
    B, T, D = c["B"], c["T"], c["D"]
    out = np.empty((B, T, D), dtype=np.float32)
    bo = np.asarray(bo, np.float32)
    for b in range(B):
        out[b] = res.results[2 * b]["o"] + res.results[2 * b + 1]["o"] + bo
    return out, res


_NC_CACHE = {}


def kernel(x, Wq, bq, Wk, bk, Wv, bv, Wo, bo):
    cfg = PROD_CFG
    key = tuple(sorted(cfg.items()))
    if key not in _NC_CACHE:
        _NC_CACHE[key] = build_nc(cfg)
    nc = _NC_CACHE[key]
    inputs = dict(Wq=Wq, bq=bq, Wk=Wk, bk=bk, Wv=Wv, bv=bv, Wo=Wo, bo=bo)
    in_maps = host_shard(cfg, np.asarray(x, np.float32), inputs)
    out, _ = run_cores(cfg, nc, in_maps, bo)
    return out
